# revision 1
# baseline (speedup 1.0000x reference)
"""Paged-attention block (QKV proj + QK-RMSNorm + partial RoPE + paged KV attention
+ o_proj) on 8 trn2 NeuronCores, tensor-parallel over heads.

Sharding: core c owns q-heads 4c..4c+3 and kv-head c (shard qkv_weight rows /
o_proj_weight columns / kv caches by head). Each core computes a partial
o_proj output; the host sums the 8 partials (the "allreduce").

All matmuls run as float32r (1 cycle/row on PE at N>=256, ~1e-4 rel err).
"""

import numpy as np

# problem constants (hardcoded per task contract)
B, SQ, HID = 4, 512, 4096
T = B * SQ
HQ, HKV, D, R = 32, 8, 128, 64
PAGE, MAX_PAGES = 64, 16
CACHED = 512
KV_LEN = CACHED + SQ          # 1024 logical kv positions per sequence
NCORES = 8
GH = HQ // NCORES             # 4 q heads per core
KB = KV_LEN // 128            # 8 kv tiles of 128
NKB = SQ // 128               # 4 new kv tiles
EPS = 1e-6
SCALE = 1.0 / float(D) ** 0.5
NEG = -1.0e30

_COMPILED = None


def _build(no_attn=False, no_oproj=False, no_qkv_mm=False, no_dma_h=False, reps=1, oproj_per_b=False):
    import concourse.tile as tile
    from concourse import mybir, bacc
    from concourse.bass import ds, ts
    from contextlib import ExitStack

    fr = mybir.dt.float32r
    f32 = mybir.dt.float32
    X = mybir.AxisListType.X

    nc = bacc.Bacc("TRN2", target_bir_lowering=False, debug=False,
                   num_devices=NCORES)

    # hidden, host-pretiled: hTb[m, p, k*128+t] = hidden[m*128+t, k*128+p]
    hT = nc.dram_tensor("hT", (T // 128, 128, HID), fr, kind="ExternalInput")
    wqkv = nc.dram_tensor("wqkv", (HID, (GH + 2) * D), fr, kind="ExternalInput")
    wo = nc.dram_tensor("wo", (GH * D, HID), fr, kind="ExternalInput")
    kcT = nc.dram_tensor("kcT", (B, D, CACHED), fr, kind="ExternalInput")
    vc = nc.dram_tensor("vc", (B, CACHED, D), fr, kind="ExternalInput")
    cosel = nc.dram_tensor("cosel", (T, R // 2), f32, kind="ExternalInput")
    sinel = nc.dram_tensor("sinel", (T, R // 2), f32, kind="ExternalInput")
    trimask = nc.dram_tensor("trimask", (128, 128), f32, kind="ExternalInput")
    mask3 = nc.dram_tensor("mask3", (128, 256), f32, kind="ExternalInput")
    ident = nc.dram_tensor("ident", (128, 128), f32, kind="ExternalInput")
    onesd = nc.dram_tensor("onesd", (128, 128), fr, kind="ExternalInput")
    outp = nc.dram_tensor("outp", (T, HID), f32, kind="ExternalOutput")

    NF = (GH + 2) * D          # 768 qkv features per core
    NQ = GH * D                # 512 (q features)
    NH = GH + 1                # 5 normed+roped heads (4 q + 1 k)

    with tile.TileContext(nc) as tc, ExitStack() as ctx:
        persist = ctx.enter_context(tc.tile_pool(name="persist", bufs=1))
        qt_pool = ctx.enter_context(tc.tile_pool(name="qt", bufs=2))
        kt_pool = ctx.enter_context(tc.tile_pool(name="kt", bufs=2))
        at_pool = ctx.enter_context(tc.tile_pool(name="at", bufs=2 if oproj_per_b else B))
        work = ctx.enter_context(tc.tile_pool(name="work", bufs=2))
        scratch = ctx.enter_context(tc.tile_pool(name="scratch", bufs=1))
        ps = ctx.enter_context(tc.tile_pool(name="ps", bufs=8, space="PSUM"))

        ident_sb = persist.tile([128, 128], f32, tag="ident")
        nc.sync.dma_start(ident_sb[:], ident[:])
        tri_sb = persist.tile([128, 128], f32, tag="tri")
        nc.sync.dma_start(tri_sb[:], trimask[:])
        m3_sb = persist.tile([128, 256], f32, tag="m3")
        nc.sync.dma_start(m3_sb[:], mask3[:])
        ones_sb = persist.tile([128, 128], fr, tag="ones")
        nc.sync.dma_start(ones_sb[:], onesd[:])
        eps_sb = persist.tile([128, 1], f32, tag="eps")
        nc.vector.memset(eps_sb[:], EPS)

        for _rep in range(reps):
            attnT = []  # per-seq [128(d), GH, 512(q)] attention outputs (o_proj lhsT)
            wo_ap = wo[:].rearrange("(ko p) f -> p ko f", p=128)

            with ExitStack() as rctx:
                if oproj_per_b:
                    opool = rctx.enter_context(tc.tile_pool(name="oproj", bufs=2))
                    outpool = rctx.enter_context(tc.tile_pool(name="outstage", bufs=2))
                qph = rctx.enter_context(tc.tile_pool(name="qkvph", bufs=1))
                hpool = rctx.enter_context(tc.tile_pool(name="hstream", bufs=3))
                # resident qkv weights [128, 32(k), 768]
                wq_sb = qph.tile([128, HID // 128, NF], fr, tag="wq")
                wq_ap = wqkv[:].rearrange("(ko p) f -> p ko f", p=128)
                for kq in range(8):
                    nc.sync.dma_start(wq_sb[:, ts(kq, 4), :], wq_ap[:, ts(kq, 4), :])

                hT_ap = hT[:].rearrange("m p (ko t) -> m p ko t", t=128)

                for b in range(B):
                    QT_b = qt_pool.tile([128, GH, SQ], fr, tag="QT")
                    KT_b = kt_pool.tile([128, SQ], fr, tag="KT")
                    V_b = kt_pool.tile([128, NKB, 128], fr, tag="Vnew")
                    kcT_b = kt_pool.tile([128, CACHED], fr, tag="kcT")
                    nc.sync.dma_start(kcT_b[:], kcT[b].rearrange("p k -> p k"))
                    vc_b = kt_pool.tile([128, NKB, 128], fr, tag="vc")
                    nc.sync.dma_start(vc_b[:], vc[b].rearrange("(blk p) d -> p blk d", p=128))

                    # Pipelined per token tile: matmuls for tile ml run
                    # first; the PE transposes for tile ml-1 are emitted after
                    # them, so PE never waits on the DVE/ACT norm+rope chain
                    # of the tile it just produced.
                    half = R // 2  # 32
                    mult = mybir.AluOpType.mult

                    def emit_transposes(ml, qkv_sb):
                        for h5 in range(NH):
                            pst = ps.tile([128, 512], f32, tag="ps", name="pst")
                            nc.tensor.transpose(pst[:, 0:128], qkv_sb[:, ts(h5, D)],
                                                ident_sb[:])
                            if h5 < GH:
                                nc.any.tensor_copy(QT_b[:, h5, ds(ml * 128, 128)],
                                                   pst[:, 0:128])
                            else:
                                nc.any.tensor_copy(KT_b[:, ds(ml * 128, 128)],
                                                   pst[:, 0:128])

                    prev = None
                    for ml in range(NKB):
                        m = b * NKB + ml
                        ht_t = hpool.tile([128, 16, 128], fr, tag="ht")
                        ht_t2 = hpool.tile([128, 16, 128], fr, tag="ht")
                        if not no_dma_h:
                            nc.sync.dma_start(ht_t[:], hT_ap[m, :, 0:16, :])
                            nc.sync.dma_start(ht_t2[:], hT_ap[m, :, 16:32, :])
                        cos_sb = work.tile([128, R // 2], f32, tag="cos", bufs=2)
                        sin_sb = work.tile([128, R // 2], f32, tag="sin", bufs=2)
                        nc.sync.dma_start(cos_sb[:], cosel[ds(m * 128, 128), :])
                        nc.sync.dma_start(sin_sb[:], sinel[ds(m * 128, 128), :])

                        # qkv projection: out [tokens(128), features(768)]
                        ps_hi = ps.tile([128, 512], f32, tag="ps")
                        ps_lo = ps.tile([128, 512], f32, tag="ps")
                        nk = HID // 128
                        if no_qkv_mm:
                            nk = 1
                        for k in range(nk):
                            src = ht_t[:, k, :] if k < 16 else ht_t2[:, k - 16, :]
                            nc.tensor.matmul(ps_hi[:], src, wq_sb[:, k, 0:512],
                                             start=(k == 0), stop=(k == nk - 1))
                            nc.tensor.matmul(ps_lo[:, 0:NF - 512], src,
                                             wq_sb[:, k, 512:NF],
                                             start=(k == 0), stop=(k == nk - 1))

                        if prev is not None:
                            emit_transposes(prev[0], prev[1])

                        # RMSNorm stats straight from PSUM
                        x2 = scratch.tile([128, NH * D], f32, tag="x2")
                        nc.scalar.square(x2[:, 0:512], ps_hi[:])
                        nc.scalar.square(x2[:, 512:NH * D], ps_lo[:, 0:128])
                        ss = work.tile([128, NH], f32, tag="ss")
                        nc.vector.reduce_sum(out=ss[:], in_=x2[:].rearrange(
                            "p (h d) -> p h d", h=NH), axis=X)
                        nc.scalar.activation(ss[:], ss[:],
                                             mybir.ActivationFunctionType.Sqrt,
                                             bias=eps_sb[:], scale=1.0 / D)
                        rstd = work.tile([128, NH], f32, tag="rstd")
                        nc.vector.reciprocal(rstd[:], ss[:])
                        # normalize PSUM -> qkv_sb (q heads + k); copy v out
                        qkv_sb = work.tile([128, NH * D], f32, tag="qkv_sb", bufs=3)
                        for h5 in range(NH):
                            src_ap = ps_hi[:, ts(h5, D)] if h5 < GH else \
                                ps_lo[:, 0:128]
                            nc.vector.tensor_scalar_mul(
                                qkv_sb[:, ts(h5, D)], src_ap, rstd[:, ds(h5, 1)])
                        nc.any.tensor_copy(V_b[:, ml, :], ps_lo[:, 128:256])

                        # rope (DVE) in place on qkv_sb
                        v3 = qkv_sb[:].rearrange("p (h d) -> p h d", h=NH)
                        x1v = v3[:, :, 0:half]
                        x2v = v3[:, :, half:R]
                        cb = cos_sb[:, None, :].to_broadcast((128, NH, half))
                        sb_ = sin_sb[:, None, :].to_broadcast((128, NH, half))
                        t1 = scratch.tile([128, NH, half], f32, tag="t1")
                        t2 = scratch.tile([128, NH, half], f32, tag="t2")
                        t3 = scratch.tile([128, NH, half], f32, tag="t3")
                        t4 = scratch.tile([128, NH, half], f32, tag="t4")
                        nc.vector.tensor_tensor(t1[:], x1v, cb, mult)
                        nc.vector.tensor_tensor(t2[:], x2v, sb_, mult)
                        nc.vector.tensor_tensor(t3[:], x1v, sb_, mult)
                        nc.vector.tensor_tensor(t4[:], x2v, cb, mult)
                        nc.vector.tensor_tensor(x1v, t1[:], t2[:],
                                                mybir.AluOpType.subtract)
                        nc.vector.tensor_tensor(x2v, t3[:], t4[:],
                                                mybir.AluOpType.add)
                        prev = (ml, qkv_sb)
                    emit_transposes(prev[0], prev[1])

                    # ---- attention for sequence b ----
                    aT = at_pool.tile([128, GH, SQ], fr, tag="attnT")
                    attnT.append(aT)
                    for h in range(0 if not no_attn else GH, GH):
                        outT_ps = ps.tile([128, 512], f32, tag="ps")
                        den_ps = ps.tile([128, 512], f32, tag="ps")
                        for t in range(KB):
                            off = 0 if t < 4 else min((t - 4) * 128, 256)
                            N = SQ - off
                            lhsT = kcT_b[:, ts(t, 128)] if t < 4 else \
                                KT_b[:, ts(t - 4, 128)]
                            vt = vc_b[:, t, :] if t < 4 else V_b[:, t - 4, :]
                            sc_ps = ps.tile([128, 512], f32, tag="ps")
                            nc.tensor.matmul(sc_ps[:, off:SQ], lhsT, QT_b[:, h, off:SQ],
                                             start=True, stop=True)
                            if t == KB - 1:
                                nc.vector.tensor_tensor(sc_ps[:, 256:512],
                                                        sc_ps[:, 256:512],
                                                        m3_sb[:], mybir.AluOpType.add)
                            elif t >= 4:
                                nc.vector.tensor_tensor(sc_ps[:, ds((t - 4) * 128, 128)],
                                                        sc_ps[:, ds((t - 4) * 128, 128)],
                                                        tri_sb[:], mybir.AluOpType.add)
                            e_t = work.tile([128, 512], fr, tag="e")
                            nc.scalar.activation(e_t[:, 0:N], sc_ps[:, off:SQ],
                                                 mybir.ActivationFunctionType.Exp,
                                                 scale=SCALE)
                            nc.tensor.matmul(outT_ps[:, off:SQ], vt, e_t[:, 0:N],
                                             start=(t == 0), stop=(t == KB - 1),
                                             skip_group_check=True)
                            nc.tensor.matmul(den_ps[:, off:SQ], ones_sb[:], e_t[:, 0:N],
                                             start=(t == 0), stop=(t == KB - 1),
                                             skip_group_check=True)
                        recip = scratch.tile([128, 512], f32, tag="recip")
                        nc.vector.reciprocal(recip[:], den_ps[:])
                        nc.vector.tensor_tensor(aT[:, h, :], outT_ps[:], recip[:],
                                                mybir.AluOpType.mult)

                    if oproj_per_b and not no_oproj:
                        for n in range(HID // 512):
                            wo_na = opool.tile([128, 2, 512], fr, tag="wo_n")
                            wo_nb = opool.tile([128, 2, 512], fr, tag="wo_n")
                            nc.sync.dma_start(wo_na[:], wo_ap[:, 0:2, ds(n * 512, 512)])
                            nc.sync.dma_start(wo_nb[:], wo_ap[:, 2:4, ds(n * 512, 512)])
                            for ml in range(NKB):
                                po = ps.tile([128, 512], f32, tag="ps")
                                for h in range(GH):
                                    wsrc = wo_na[:, h, :] if h < 2 else wo_nb[:, h - 2, :]
                                    nc.tensor.matmul(po[:], aT[:, h, ts(ml, 128)],
                                                     wsrc,
                                                     start=(h == 0), stop=(h == GH - 1))
                                ob = outpool.tile([128, 512], f32, tag="ob")
                                if (ml + n) % 2 == 0:
                                    nc.vector.tensor_copy(ob[:], po[:])
                                else:
                                    nc.scalar.copy(ob[:], po[:])
                                nc.sync.dma_start(
                                    outp[ds((b * NKB + ml) * 128, 128),
                                         ds(n * 512, 512)], ob[:])

            # ---- o_proj (phase-3 variant): partial = attnT.T @ woT ----
            if oproj_per_b:
                continue
            with tc.tile_pool(name="oproj", bufs=2) as opool, \
                 tc.tile_pool(name="outstage", bufs=3) as outpool:
                for n in range(0 if not no_oproj else HID // 512, HID // 512):
                    wo_n = opool.tile([128, GH, 512], fr, tag="wo_n")
                    nc.sync.dma_start(wo_n[:], wo_ap[:, :, ds(n * 512, 512)])
                    for b in range(B):
                        for ml in range(NKB):
                            po = ps.tile([128, 512], f32, tag="ps")
                            for h in range(GH):
                                nc.tensor.matmul(po[:], attnT[b][:, h, ts(ml, 128)],
                                                 wo_n[:, h, :],
                                                 start=(h == 0), stop=(h == GH - 1))
                            ob = outpool.tile([128, 512], f32, tag="ob")
                            if (b * NKB + ml) % 2 == 0:
                                nc.vector.tensor_copy(ob[:], po[:])
                            else:
                                nc.scalar.copy(ob[:], po[:])
                            nc.sync.dma_start(
                                outp[ds((b * NKB + ml) * 128, 128), ds(n * 512, 512)],
                                ob[:])

    nc.compile()
    return nc


def _get_compiled():
    global _COMPILED
    if _COMPILED is None:
        _COMPILED = _build()
    return _COMPILED


def _prep_inputs(hidden_states, cos, sin, positions, k_cache, v_cache, page_table,
                 cache_seqlens, cu_seqlens_q, qkv_weight, o_proj_weight,
                 q_norm_weight, k_norm_weight):
    f32 = np.float32
    pos = np.asarray(positions).reshape(B, SQ)
    assert np.array_equal(np.asarray(cache_seqlens),
                          np.full(B, CACHED, np.int32)), "cache_seqlens != CACHED"
    assert np.array_equal(np.asarray(cu_seqlens_q),
                          np.arange(B + 1, dtype=np.int64) * SQ), "cu_seqlens ragged"
    assert (pos == CACHED + np.arange(SQ)[None, :]).all(), "positions ragged"
    assert np.allclose(q_norm_weight, 1.0) and np.allclose(k_norm_weight, 1.0), \
        "non-unit norm weights unsupported"

    pt = np.asarray(page_table)
    phys = (pt[:, :, None] * PAGE + np.arange(PAGE)[None, None, :]).reshape(B, -1)
    slots = pt[np.arange(B)[:, None], pos // PAGE] * PAGE + pos % PAGE
    assert np.array_equal(slots, phys[:, CACHED:]), "non-append page layout"

    kf = np.asarray(k_cache).reshape(-1, HKV, D)
    vf = np.asarray(v_cache).reshape(-1, HKV, D)
    Kc = kf[phys[:, :CACHED]]          # [B, 512, HKV, D]
    Vc = vf[phys[:, :CACHED]]

    cos_sel = np.ascontiguousarray(np.asarray(cos)[positions], dtype=f32)
    sin_sel = np.ascontiguousarray(np.asarray(sin)[positions], dtype=f32)
    # hTb[m, p, k*128+t] = hidden[m*128+t, k*128+p]
    hT = np.ascontiguousarray(
        np.asarray(hidden_states, dtype=f32).reshape(T // 128, 128, HID // 128, 128)
        .transpose(0, 3, 2, 1).reshape(T // 128, 128, HID))
    tri = np.where(np.arange(128)[None, :] >= np.arange(128)[:, None],
                   np.float32(0.0), np.float32(NEG))
    m3 = np.concatenate([np.full((128, 128), NEG, f32), tri], axis=1)
    eye = np.eye(128, dtype=f32)

    qw = np.asarray(qkv_weight)
    ow = np.asarray(o_proj_weight)
    in_maps = []
    for c in range(NCORES):
        rows = np.concatenate([
            qw[c * GH * D:(c + 1) * GH * D],
            qw[HQ * D + c * D: HQ * D + (c + 1) * D],
            qw[HQ * D + HKV * D + c * D: HQ * D + HKV * D + (c + 1) * D],
        ], axis=0)
        in_maps.append(dict(
            hT=hT,
            wqkv=np.ascontiguousarray(rows.T, dtype=f32),
            wo=np.ascontiguousarray(ow[:, c * GH * D:(c + 1) * GH * D].T, dtype=f32),
            kcT=np.ascontiguousarray(Kc[:, :, c, :].transpose(0, 2, 1), dtype=f32),
            vc=np.ascontiguousarray(Vc[:, :, c, :], dtype=f32),
            cosel=cos_sel, sinel=sin_sel, trimask=tri, ident=eye, mask3=m3,
            onesd=np.ones((128, 128), dtype=f32),
        ))
    return in_maps


def kernel(**inputs) -> np.ndarray:
    from concourse.bass_utils import run_bass_kernel_spmd
    in_maps = _prep_inputs(**inputs)
    nc = _get_compiled()
    res = run_bass_kernel_spmd(nc, in_maps, core_ids=list(range(NCORES)))
    acc = res.results[0]["outp"].astype(np.float32).copy()
    for c in range(1, NCORES):
        acc += res.results[c]["outp"]
    return acc



# revision 11
# speedup vs baseline: 1.0096x; 1.0096x over previous
"""Paged-attention block (QKV proj + QK-RMSNorm + partial RoPE + paged KV attention
+ o_proj) on 8 trn2 NeuronCores, tensor-parallel over heads.

Sharding: core c owns q-heads 4c..4c+3 and kv-head c (shard qkv_weight rows /
o_proj_weight columns / kv caches by head). Each core computes a partial
o_proj output; the host sums the 8 partials (the "allreduce").

v2: fp16 matmul operands end-to-end (same 1 cyc/row PE throughput as f32r,
half the DMA/SBUF traffic, 2x DVE modes), weights fully SBUF-resident with
DMA issue order prioritizing the first tile's operands, per-sequence software
pipeline (attention of seq b-1 between the QKV tiles of seq b, o_proj of b-1
at the end of seq b), depth-2 pipelined attention inner loop so the PE never
waits on the exp() chain, exact causal windows, merged per-sequence output DMA.
"""

import numpy as np

# problem constants (hardcoded per task contract)
B, SQ, HID = 4, 512, 4096
T = B * SQ
HQ, HKV, D, R = 32, 8, 128, 64
PAGE, MAX_PAGES = 64, 16
CACHED = 512
KV_LEN = CACHED + SQ          # 1024 logical kv positions per sequence
NCORES = 8
GH = HQ // NCORES             # 4 q heads per core
KB = KV_LEN // 128            # 8 kv tiles of 128
NKB = SQ // 128               # 4 new kv tiles
EPS = 1e-6
SCALE = 1.0 / float(D) ** 0.5
NEG = -1.0e30
EXP_BIAS = -4.0               # keeps exp() outputs inside fp16 range

_COMPILED = None


def _build(reps=1):
    import concourse.tile as tile
    from concourse import mybir, bacc
    from concourse.bass import ds, ts
    from contextlib import ExitStack

    f16 = mybir.dt.float16
    f32 = mybir.dt.float32
    mult = mybir.AluOpType.mult
    add = mybir.AluOpType.add

    nc = bacc.Bacc("TRN2", target_bir_lowering=False, debug=False,
                   num_devices=NCORES)

    NF = (GH + 2) * D          # 768 qkv features per core
    NH = GH + 1                # 5 normed+roped heads (4 q + 1 k)
    half = R // 2              # 32
    NT = T // 128              # 16 token tiles

    # hidden, host-pretiled: hT[m, p, k*128+t] = hidden[m*128+t, k*128+p]
    hT = nc.dram_tensor("hT", (NT, 128, HID), f16, kind="ExternalInput")
    wqkv = nc.dram_tensor("wqkv", (HID, NF), f16, kind="ExternalInput")
    wo = nc.dram_tensor("wo", (GH * D, HID), f16, kind="ExternalInput")
    kcT = nc.dram_tensor("kcT", (B, D, CACHED), f16, kind="ExternalInput")
    vc = nc.dram_tensor("vc", (B, CACHED, D), f16, kind="ExternalInput")
    # cs[p, m, :] = concat(cos, sin) at token m*128+p   [128, NT, 64] fp16
    csel = nc.dram_tensor("csel", (128, NT, R), f16, kind="ExternalInput")
    trimask = nc.dram_tensor("trimask", (128, 128), f32, kind="ExternalInput")
    ident = nc.dram_tensor("ident", (128, 128), f16, kind="ExternalInput")
    onesd = nc.dram_tensor("onesd", (128, 128), f16, kind="ExternalInput")
    outp = nc.dram_tensor("outp", (T, HID), f16, kind="ExternalOutput")

    with tile.TileContext(nc) as tc, ExitStack() as ctx:
        persist = ctx.enter_context(tc.tile_pool(name="persist", bufs=1))
        qt_pool = ctx.enter_context(tc.tile_pool(name="qt", bufs=2))
        kt_pool = ctx.enter_context(tc.tile_pool(name="kt", bufs=2))
        at_pool = ctx.enter_context(tc.tile_pool(name="at", bufs=B))
        work = ctx.enter_context(tc.tile_pool(name="work", bufs=2))
        scratch = ctx.enter_context(tc.tile_pool(name="scratch", bufs=1))
        hpool = ctx.enter_context(tc.tile_pool(name="hstream", bufs=5))
        outpool = ctx.enter_context(tc.tile_pool(name="outstage", bufs=1))
        ps = ctx.enter_context(tc.tile_pool(name="ps", bufs=8, space="PSUM"))

        ident_sb = persist.tile([128, 128], f16, tag="ident")
        nc.sync.dma_start(ident_sb[:], ident[:])
        tri_sb = persist.tile([128, 128], f32, tag="tri")
        nc.sync.dma_start(tri_sb[:], trimask[:])
        ones_sb = persist.tile([128, 128], f16, tag="ones")
        nc.sync.dma_start(ones_sb[:], onesd[:])
        eps_sb = persist.tile([128, 1], f32, tag="eps")
        nc.vector.memset(eps_sb[:], EPS)
        eb_sb = persist.tile([128, 1], f32, tag="eb")
        nc.vector.memset(eb_sb[:], EXP_BIAS)
        cs_all = persist.tile([128, NT, R], f16, tag="cs")
        nc.sync.dma_start(cs_all[:], csel[:])

        for _rep in range(reps):
            with ExitStack() as rctx:
                wpool = rctx.enter_context(tc.tile_pool(name="wres", bufs=1))
                wq_sb = wpool.tile([128, HID // 128, NF], f16, tag="wq")
                wq_ap = wqkv[:].rearrange("(ko p) f -> p ko f", p=128)
                wo_sb = wpool.tile([128, GH, HID], f16, tag="wo")
                wo_ap = wo[:].rearrange("(ko p) f -> p ko f", p=128)
                hT_ap = hT[:].rearrange("m p (ko t) -> m p ko t", t=128)

                hts = {}

                def emit_ht_dma(m):
                    ht_t = hpool.tile([128, HID // 128, 128], f16, tag="ht")
                    nc.sync.dma_start(ht_t[:], hT_ap[m])
                    hts[m] = ht_t

                # issue order = transfer order; everything must be EMITTED
                # before its first reader, so all wq chunks go out up front,
                # interleaved with the first hidden tiles. wo chunks drip in
                # during the first 4 tile slots (first o_proj reads at m=7).
                emit_ht_dma(0)
                nc.sync.dma_start(wq_sb[:, 0:4, :], wq_ap[:, 0:4, :])
                nc.sync.dma_start(wq_sb[:, 4:8, :], wq_ap[:, 4:8, :])
                emit_ht_dma(1)
                nc.sync.dma_start(wq_sb[:, 8:12, :], wq_ap[:, 8:12, :])
                nc.sync.dma_start(wq_sb[:, 12:16, :], wq_ap[:, 12:16, :])
                emit_ht_dma(2)
                nc.sync.dma_start(wq_sb[:, 16:20, :], wq_ap[:, 16:20, :])
                nc.sync.dma_start(wq_sb[:, 20:24, :], wq_ap[:, 20:24, :])
                emit_ht_dma(3)
                nc.sync.dma_start(wq_sb[:, 24:28, :], wq_ap[:, 24:28, :])
                nc.sync.dma_start(wq_sb[:, 28:32, :], wq_ap[:, 28:32, :])
                wdmas = [lambda kq=kq: nc.sync.dma_start(
                    wo_sb[:, kq, :], wo_ap[:, kq, :]) for kq in range(GH)]

                seq = {}    # per-seq tiles
                attnT = {}  # per-seq o_proj lhsT tiles

                def emit_transposes(b, ml, qkv_sb):
                    s = seq[b]
                    for h5 in range(NH):
                        pst = ps.tile([128, 512], f16, tag="ps", name="pst")
                        nc.tensor.transpose(pst[:, 0:128], qkv_sb[:, ts(h5, D)],
                                            ident_sb[:])
                        if h5 < GH:
                            nc.any.tensor_copy(s["QT"][:, h5, ds(ml * 128, 128)],
                                               pst[:, 0:128])
                        else:
                            nc.any.tensor_copy(s["KT"][:, ds(ml * 128, 128)],
                                               pst[:, 0:128])

                def emit_attention(b):
                    s = seq[b]
                    aT = at_pool.tile([128, GH, SQ], f16, tag="attnT")
                    attnT[b] = aT

                    def lhsT(t):
                        return s["kcT"][:, ts(t, 128)] if t < 4 else \
                            s["KT"][:, ts(t - 4, 128)]

                    def off_of(t):
                        return 0 if t < 4 else (t - 4) * 128

                    for h in range(GH):
                        outT_ps = ps.tile([128, 512], f32, tag="ps")
                        den_ps = ps.tile([128, 512], f32, tag="ps")
                        scs, es = {}, {}

                        def emit_scores(t):
                            off = off_of(t)
                            sc_ps = ps.tile([128, 512], f32, tag="ps")
                            nc.tensor.matmul(sc_ps[:, off:SQ], lhsT(t),
                                             s["QT"][:, h, off:SQ],
                                             start=True, stop=True)
                            scs[t] = sc_ps

                        def emit_exp(t):
                            off = off_of(t)
                            if t >= 4:
                                nc.vector.tensor_tensor(
                                    scs[t][:, ds(off, 128)],
                                    scs[t][:, ds(off, 128)], tri_sb[:], add)
                            e_t = work.tile([128, 512], f16, tag="e")
                            nc.scalar.activation(
                                e_t[:, 0:SQ - off], scs[t][:, off:SQ],
                                mybir.ActivationFunctionType.Exp,
                                bias=eb_sb[:], scale=SCALE)
                            es[t] = e_t

                        def emit_pv(t):
                            off = off_of(t)
                            N = SQ - off
                            vt = s["vc"][:, t, :] if t < 4 else \
                                s["V"][:, t - 4, :]
                            nc.tensor.matmul(outT_ps[:, off:SQ], vt,
                                             es[t][:, 0:N],
                                             start=(t == 0), stop=(t == KB - 1),
                                             skip_group_check=True)
                            nc.tensor.matmul(den_ps[:, off:SQ], ones_sb[:],
                                             es[t][:, 0:N],
                                             start=(t == 0), stop=(t == KB - 1),
                                             skip_group_check=True)

                        # depth-2 pipeline: scores run two kv-tiles ahead of
                        # the exp -> PV/den consumers so PE never waits on ACT
                        emit_scores(0)
                        emit_exp(0)
                        emit_scores(1)
                        for t in range(KB):
                            if t + 2 < KB:
                                emit_scores(t + 2)
                            if t + 1 < KB:
                                emit_exp(t + 1)
                            emit_pv(t)
                        recip = scratch.tile([128, 512], f32, tag="recip")
                        nc.vector.reciprocal(recip[:], den_ps[:])
                        nc.vector.tensor_tensor(aT[:, h, :], outT_ps[:], recip[:],
                                                mult)

                def emit_oproj(b):
                    aT = attnT[b]
                    ob = outpool.tile([128, NKB, HID], f16, tag="ob")
                    for n in range(HID // 512):
                        for ml in range(NKB):
                            po = ps.tile([128, 512], f32, tag="ps")
                            for h in range(GH):
                                nc.tensor.matmul(po[:], aT[:, h, ts(ml, 128)],
                                                 wo_sb[:, h, ds(n * 512, 512)],
                                                 start=(h == 0), stop=(h == GH - 1))
                            if (n + ml) % 2 == 0:
                                nc.vector.tensor_copy(
                                    ob[:, ml, ds(n * 512, 512)], po[:])
                            else:
                                nc.scalar.copy(
                                    ob[:, ml, ds(n * 512, 512)], po[:])
                    nc.sync.dma_start(
                        outp[ds(b * SQ, SQ), :].rearrange(
                            "(ml p) f -> p ml f", p=128), ob[:])

                pending_T = None
                for m in range(NT):
                    b, ml = divmod(m, NKB)
                    if ml == 0:
                        QT_b = qt_pool.tile([128, GH, SQ], f16, tag="QT")
                        KT_b = kt_pool.tile([128, SQ], f16, tag="KT")
                        V_b = kt_pool.tile([128, NKB, 128], f16, tag="Vnew")
                        kcT_b = kt_pool.tile([128, CACHED], f16, tag="kcT")
                        nc.sync.dma_start(kcT_b[:], kcT[b].rearrange("p k -> p k"))
                        vc_b = kt_pool.tile([128, NKB, 128], f16, tag="vc")
                        nc.sync.dma_start(
                            vc_b[:], vc[b].rearrange("(blk p) d -> p blk d", p=128))
                        seq[b] = dict(QT=QT_b, KT=KT_b, V=V_b, kcT=kcT_b, vc=vc_b)

                    if wdmas:
                        wdmas.pop(0)()
                    if m + 4 < NT:
                        emit_ht_dma(m + 4)

                    # qkv projection: out [tokens(128), features(768)]
                    ht_t = hts.pop(m)
                    ps_hi = ps.tile([128, 512], f32, tag="ps")
                    ps_lo = ps.tile([128, 512], f32, tag="ps")
                    for k in range(HID // 128):
                        nc.tensor.matmul(ps_hi[:], ht_t[:, k, :],
                                         wq_sb[:, k, 0:512],
                                         start=(k == 0), stop=(k == 31))
                        nc.tensor.matmul(ps_lo[:, 0:NF - 512], ht_t[:, k, :],
                                         wq_sb[:, k, 512:NF],
                                         start=(k == 0), stop=(k == 31))

                    if pending_T is not None:
                        emit_transposes(*pending_T)

                    # RMSNorm stats straight from PSUM
                    x2 = scratch.tile([128, NH * D], f32, tag="x2")
                    nc.scalar.square(x2[:, 0:GH * D], ps_hi[:])
                    nc.scalar.square(x2[:, GH * D:NH * D], ps_lo[:, 0:128])
                    ss = work.tile([128, NH], f32, tag="ss")
                    nc.vector.reduce_sum(out=ss[:], in_=x2[:].rearrange(
                        "p (h d) -> p h d", h=NH), axis=mybir.AxisListType.X)
                    nc.scalar.activation(ss[:], ss[:],
                                         mybir.ActivationFunctionType.Sqrt,
                                         bias=eps_sb[:], scale=1.0 / D)
                    rstd = work.tile([128, NH], f32, tag="rstd")
                    nc.vector.reciprocal(rstd[:], ss[:])
                    # normalize PSUM -> qkv_sb fp16 (q heads + k); copy v out
                    qkv_sb = work.tile([128, NH * D], f16, tag="qkv_sb", bufs=3)
                    for h5 in range(NH):
                        src_ap = ps_hi[:, ts(h5, D)] if h5 < GH else \
                            ps_lo[:, 0:128]
                        nc.vector.tensor_scalar_mul(
                            qkv_sb[:, ts(h5, D)], src_ap, rstd[:, ds(h5, 1)])
                    nc.any.tensor_copy(seq[b]["V"][:, ml, :], ps_lo[:, 128:256])

                    # partial rope (DVE, all-fp16) in place on qkv_sb
                    v3 = qkv_sb[:].rearrange("p (h d) -> p h d", h=NH)
                    x1v = v3[:, :, 0:half]
                    x2v = v3[:, :, half:R]
                    cb = cs_all[:, None, m, 0:half].to_broadcast((128, NH, half))
                    sb_ = cs_all[:, None, m, half:R].to_broadcast((128, NH, half))
                    t1 = scratch.tile([128, NH, half], f16, tag="t1")
                    t2 = scratch.tile([128, NH, half], f16, tag="t2")
                    t3 = scratch.tile([128, NH, half], f16, tag="t3")
                    t4 = scratch.tile([128, NH, half], f16, tag="t4")
                    nc.vector.tensor_tensor(t1[:], x1v, cb, mult)
                    nc.vector.tensor_tensor(t2[:], x2v, sb_, mult)
                    nc.vector.tensor_tensor(t3[:], x1v, sb_, mult)
                    nc.vector.tensor_tensor(t4[:], x2v, cb, mult)
                    nc.vector.tensor_tensor(x1v, t1[:], t2[:],
                                            mybir.AluOpType.subtract)
                    nc.vector.tensor_tensor(x2v, t3[:], t4[:], add)
                    pending_T = (b, ml, qkv_sb)

                    # pipeline: attention of seq b-1 after the first tile of
                    # seq b (its transposes just ran); o_proj of b-1 after the
                    # last tile so the kernel tail stays short
                    if ml == 0 and b > 0:
                        emit_attention(b - 1)
                    if ml == NKB - 1 and b > 0:
                        emit_oproj(b - 1)

                emit_transposes(*pending_T)
                emit_attention(B - 1)
                emit_oproj(B - 1)

    nc.compile()
    return nc


def _get_compiled():
    global _COMPILED
    if _COMPILED is None:
        _COMPILED = _build()
    return _COMPILED


def _prep_inputs(hidden_states, cos, sin, positions, k_cache, v_cache, page_table,
                 cache_seqlens, cu_seqlens_q, qkv_weight, o_proj_weight,
                 q_norm_weight, k_norm_weight):
    f16 = np.float16
    pos = np.asarray(positions).reshape(B, SQ)
    assert np.array_equal(np.asarray(cache_seqlens),
                          np.full(B, CACHED, np.int32)), "cache_seqlens != CACHED"
    assert np.array_equal(np.asarray(cu_seqlens_q),
                          np.arange(B + 1, dtype=np.int64) * SQ), "cu_seqlens ragged"
    assert (pos == CACHED + np.arange(SQ)[None, :]).all(), "positions ragged"
    assert np.allclose(q_norm_weight, 1.0) and np.allclose(k_norm_weight, 1.0), \
        "non-unit norm weights unsupported"

    pt = np.asarray(page_table)
    phys = (pt[:, :, None] * PAGE + np.arange(PAGE)[None, None, :]).reshape(B, -1)
    slots = pt[np.arange(B)[:, None], pos // PAGE] * PAGE + pos % PAGE
    assert np.array_equal(slots, phys[:, CACHED:]), "non-append page layout"

    kf = np.asarray(k_cache).reshape(-1, HKV, D)
    vf = np.asarray(v_cache).reshape(-1, HKV, D)
    Kc = kf[phys[:, :CACHED]]          # [B, 512, HKV, D]
    Vc = vf[phys[:, :CACHED]]

    # cs[p, m, :] = concat(cos, sin)[token m*128+p]
    cs = np.concatenate([np.asarray(cos)[positions], np.asarray(sin)[positions]],
                        axis=1).astype(f16).reshape(T // 128, 128, R)
    cs = np.ascontiguousarray(cs.transpose(1, 0, 2))
    # hT[m, p, k*128+t] = hidden[m*128+t, k*128+p]
    hT = np.ascontiguousarray(
        np.asarray(hidden_states, dtype=f16).reshape(T // 128, 128, HID // 128, 128)
        .transpose(0, 3, 2, 1).reshape(T // 128, 128, HID))
    tri = np.where(np.arange(128)[None, :] >= np.arange(128)[:, None],
                   np.float32(0.0), np.float32(NEG))
    eye = np.eye(128, dtype=f16)

    qw = np.asarray(qkv_weight)
    ow = np.asarray(o_proj_weight)
    in_maps = []
    for c in range(NCORES):
        rows = np.concatenate([
            qw[c * GH * D:(c + 1) * GH * D],
            qw[HQ * D + c * D: HQ * D + (c + 1) * D],
            qw[HQ * D + HKV * D + c * D: HQ * D + HKV * D + (c + 1) * D],
        ], axis=0)
        in_maps.append(dict(
            hT=hT,
            wqkv=np.ascontiguousarray(rows.T, dtype=f16),
            wo=np.ascontiguousarray(ow[:, c * GH * D:(c + 1) * GH * D].T, dtype=f16),
            kcT=np.ascontiguousarray(Kc[:, :, c, :].transpose(0, 2, 1), dtype=f16),
            vc=np.ascontiguousarray(Vc[:, :, c, :], dtype=f16),
            csel=cs, trimask=tri, ident=eye,
            onesd=np.ones((128, 128), dtype=f16),
        ))
    return in_maps


def kernel(**inputs) -> np.ndarray:
    from concourse.bass_utils import run_bass_kernel_spmd
    in_maps = _prep_inputs(**inputs)
    nc = _get_compiled()
    res = run_bass_kernel_spmd(nc, in_maps, core_ids=list(range(NCORES)))
    acc = res.results[0]["outp"].astype(np.float32)
    for c in range(1, NCORES):
        acc += res.results[c]["outp"].astype(np.float32)
    return acc


# revision 17
# speedup vs baseline: 172.3686x; 170.7328x over previous
"""Paged-attention block (QKV proj + QK-RMSNorm + partial RoPE + paged KV attention
+ o_proj) on 8 trn2 NeuronCores, tensor-parallel over heads.

Sharding: core c owns q-heads 4c..4c+3 and kv-head c (shard qkv_weight rows /
o_proj_weight columns / kv caches by head). Each core computes a partial
o_proj output; the host sums the 8 partials (the "allreduce").

v2: fp16 matmul operands end-to-end (same 1 cyc/row PE throughput as f32r,
half the DMA/SBUF traffic, 2x DVE modes), weights fully SBUF-resident with
DMA issue order prioritizing the first tile's operands, per-sequence software
pipeline (attention of seq b-1 between the QKV tiles of seq b, o_proj of b-1
at the end of seq b), depth-2 pipelined attention inner loop so the PE never
waits on the exp() chain, exact causal windows, merged per-sequence output DMA.
"""

import numpy as np

# problem constants (hardcoded per task contract)
B, SQ, HID = 4, 512, 4096
T = B * SQ
HQ, HKV, D, R = 32, 8, 128, 64
PAGE, MAX_PAGES = 64, 16
CACHED = 512
KV_LEN = CACHED + SQ          # 1024 logical kv positions per sequence
NCORES = 8
GH = HQ // NCORES             # 4 q heads per core
KB = KV_LEN // 128            # 8 kv tiles of 128
NKB = SQ // 128               # 4 new kv tiles
EPS = 1e-6
SCALE = 1.0 / float(D) ** 0.5
NEG = -1.0e30
EXP_BIAS = -4.0               # keeps exp() outputs inside fp16 range

_COMPILED = None


def _build(reps=1):
    import concourse.tile as tile
    from concourse import mybir, bacc
    from concourse.bass import ds, ts
    from contextlib import ExitStack

    f16 = mybir.dt.float16
    f32 = mybir.dt.float32
    mult = mybir.AluOpType.mult
    add = mybir.AluOpType.add

    nc = bacc.Bacc("TRN2", target_bir_lowering=False, debug=False,
                   num_devices=NCORES)

    NF = (GH + 2) * D          # 768 qkv features per core
    NH = GH + 1                # 5 normed+roped heads (4 q + 1 k)
    half = R // 2              # 32
    NT = T // 128              # 16 token tiles

    # hidden, host-pretiled: hT[m, p, k*128+t] = hidden[m*128+t, k*128+p]
    hT = nc.dram_tensor("hT", (NT, 128, HID), f16, kind="ExternalInput")
    wqkv = nc.dram_tensor("wqkv", (HID, NF), f16, kind="ExternalInput")
    wo = nc.dram_tensor("wo", (GH * D, HID), f16, kind="ExternalInput")
    kcT = nc.dram_tensor("kcT", (B, D, CACHED), f16, kind="ExternalInput")
    vc = nc.dram_tensor("vc", (B, CACHED, D), f16, kind="ExternalInput")
    # cs[p, m, :] = concat(cos, sin) at token m*128+p   [128, NT, 64] fp16
    csel = nc.dram_tensor("csel", (128, NT, R), f16, kind="ExternalInput")
    trimask = nc.dram_tensor("trimask", (128, 128), f32, kind="ExternalInput")
    ident = nc.dram_tensor("ident", (128, 128), f16, kind="ExternalInput")
    onesd = nc.dram_tensor("onesd", (128, 128), f16, kind="ExternalInput")
    outp = nc.dram_tensor("outp", (T, HID), f16, kind="ExternalOutput")

    with tile.TileContext(nc) as tc, ExitStack() as ctx:
        persist = ctx.enter_context(tc.tile_pool(name="persist", bufs=1))
        qt_pool = ctx.enter_context(tc.tile_pool(name="qt", bufs=2))
        kt_pool = ctx.enter_context(tc.tile_pool(name="kt", bufs=2))
        at_pool = ctx.enter_context(tc.tile_pool(name="at", bufs=B))
        work = ctx.enter_context(tc.tile_pool(name="work", bufs=2))
        scratch = ctx.enter_context(tc.tile_pool(name="scratch", bufs=1))
        hpool = ctx.enter_context(tc.tile_pool(name="hstream", bufs=5))
        outpool = ctx.enter_context(tc.tile_pool(name="outstage", bufs=2))
        ps = ctx.enter_context(tc.tile_pool(name="ps", bufs=8, space="PSUM"))

        ident_sb = persist.tile([128, 128], f16, tag="ident")
        nc.sync.dma_start(ident_sb[:], ident[:])
        tri_sb = persist.tile([128, 128], f32, tag="tri")
        nc.sync.dma_start(tri_sb[:], trimask[:])
        ones_sb = persist.tile([128, 128], f16, tag="ones")
        nc.sync.dma_start(ones_sb[:], onesd[:])
        eps_sb = persist.tile([128, 1], f32, tag="eps")
        nc.vector.memset(eps_sb[:], EPS)
        eb_sb = persist.tile([128, 1], f32, tag="eb")
        nc.vector.memset(eb_sb[:], EXP_BIAS)
        cs_all = persist.tile([128, NT, R], f16, tag="cs")
        nc.sync.dma_start(cs_all[:], csel[:])

        for _rep in range(reps):
            with ExitStack() as rctx:
                wpool = rctx.enter_context(tc.tile_pool(name="wres", bufs=1))
                wq_sb = wpool.tile([128, HID // 128, NF], f16, tag="wq")
                wq_ap = wqkv[:].rearrange("(ko p) f -> p ko f", p=128)
                wo_sb = wpool.tile([128, GH, HID], f16, tag="wo")
                wo_ap = wo[:].rearrange("(ko p) f -> p ko f", p=128)
                hT_ap = hT[:].rearrange("m p (ko t) -> m p ko t", t=128)

                hts = {}

                def emit_ht_dma(m, split=False):
                    ht_t = hpool.tile([128, HID // 128, 128], f16, tag="ht")
                    if split:
                        nc.sync.dma_start(ht_t[:, 0:8, :], hT_ap[m, :, 0:8])
                        nc.sync.dma_start(ht_t[:, 8:16, :], hT_ap[m, :, 8:16])
                        nc.sync.dma_start(ht_t[:, 16:32, :], hT_ap[m, :, 16:32])
                    else:
                        nc.sync.dma_start(ht_t[:], hT_ap[m])
                    hts[m] = ht_t

                # issue order = transfer order; everything must be EMITTED
                # before its first reader, so all wq chunks go out up front,
                # interleaved with the first hidden tiles. wo chunks drip in
                # during the first 4 tile slots (first o_proj reads at m=7).
                emit_ht_dma(0, split=True)
                nc.sync.dma_start(wq_sb[:, 0:4, :], wq_ap[:, 0:4, :])
                nc.sync.dma_start(wq_sb[:, 4:8, :], wq_ap[:, 4:8, :])
                emit_ht_dma(1)
                nc.sync.dma_start(wq_sb[:, 8:12, :], wq_ap[:, 8:12, :])
                nc.sync.dma_start(wq_sb[:, 12:16, :], wq_ap[:, 12:16, :])
                emit_ht_dma(2)
                nc.sync.dma_start(wq_sb[:, 16:20, :], wq_ap[:, 16:20, :])
                nc.sync.dma_start(wq_sb[:, 20:24, :], wq_ap[:, 20:24, :])
                emit_ht_dma(3)
                nc.sync.dma_start(wq_sb[:, 24:28, :], wq_ap[:, 24:28, :])
                nc.sync.dma_start(wq_sb[:, 28:32, :], wq_ap[:, 28:32, :])
                wdmas = [lambda kq=kq: nc.sync.dma_start(
                    wo_sb[:, kq, :], wo_ap[:, kq, :]) for kq in range(GH)]

                seq = {}    # per-seq tiles
                attnT = {}  # per-seq o_proj lhsT tiles

                def emit_transposes(b, ml, qkv_sb):
                    s = seq[b]
                    for h5 in range(NH):
                        pst = ps.tile([128, 512], f16, tag="ps", name="pst")
                        nc.tensor.transpose(pst[:, 0:128], qkv_sb[:, ts(h5, D)],
                                            ident_sb[:])
                        if h5 < GH:
                            nc.any.tensor_copy(s["QT"][:, h5, ds(ml * 128, 128)],
                                               pst[:, 0:128])
                        else:
                            nc.any.tensor_copy(s["KT"][:, ds(ml * 128, 128)],
                                               pst[:, 0:128])

                def emit_attention(b):
                    s = seq[b]
                    aT = at_pool.tile([128, GH, SQ], f16, tag="attnT")
                    attnT[b] = aT

                    def lhsT(t):
                        return s["kcT"][:, ts(t, 128)] if t < 4 else \
                            s["KT"][:, ts(t - 4, 128)]

                    def off_of(t):
                        return 0 if t < 4 else (t - 4) * 128

                    for h in range(GH):
                        outT_ps = ps.tile([128, 512], f32, tag="ps")
                        den_ps = ps.tile([128, 512], f32, tag="ps")
                        scs, es = {}, {}

                        def emit_scores(t):
                            off = off_of(t)
                            sc_ps = ps.tile([128, 512], f32, tag="ps")
                            nc.tensor.matmul(sc_ps[:, off:SQ], lhsT(t),
                                             s["QT"][:, h, off:SQ],
                                             start=True, stop=True)
                            scs[t] = sc_ps

                        def emit_exp(t):
                            off = off_of(t)
                            if t >= 4:
                                nc.vector.tensor_tensor(
                                    scs[t][:, ds(off, 128)],
                                    scs[t][:, ds(off, 128)], tri_sb[:], add)
                            e_t = work.tile([128, 512], f16, tag="e")
                            nc.scalar.activation(
                                e_t[:, 0:SQ - off], scs[t][:, off:SQ],
                                mybir.ActivationFunctionType.Exp,
                                bias=eb_sb[:], scale=SCALE)
                            es[t] = e_t

                        def emit_pv(t):
                            off = off_of(t)
                            N = SQ - off
                            vt = s["vc"][:, t, :] if t < 4 else \
                                s["V"][:, t - 4, :]
                            nc.tensor.matmul(outT_ps[:, off:SQ], vt,
                                             es[t][:, 0:N],
                                             start=(t == 0), stop=(t == KB - 1),
                                             skip_group_check=True)
                            nc.tensor.matmul(den_ps[:, off:SQ], ones_sb[:],
                                             es[t][:, 0:N],
                                             start=(t == 0), stop=(t == KB - 1),
                                             skip_group_check=True)

                        # depth-2 pipeline: scores run two kv-tiles ahead of
                        # the exp -> PV/den consumers so PE never waits on ACT
                        emit_scores(0)
                        emit_exp(0)
                        emit_scores(1)
                        for t in range(KB):
                            if t + 2 < KB:
                                emit_scores(t + 2)
                            if t + 1 < KB:
                                emit_exp(t + 1)
                            emit_pv(t)
                        recip = scratch.tile([128, 512], f32, tag="recip")
                        nc.vector.reciprocal(recip[:], den_ps[:])
                        nc.vector.tensor_tensor(aT[:, h, :], outT_ps[:], recip[:],
                                                mult)

                def emit_oproj(b):
                    aT = attnT[b]
                    for ml in range(NKB):
                        ob = outpool.tile([128, HID], f16, tag="ob")
                        for n in range(HID // 512):
                            po = ps.tile([128, 512], f32, tag="ps")
                            for h in range(GH):
                                nc.tensor.matmul(po[:], aT[:, h, ts(ml, 128)],
                                                 wo_sb[:, h, ds(n * 512, 512)],
                                                 start=(h == 0), stop=(h == GH - 1))
                            if (n + ml) % 2 == 0:
                                nc.vector.tensor_copy(ob[:, ds(n * 512, 512)],
                                                      po[:])
                            else:
                                nc.scalar.copy(ob[:, ds(n * 512, 512)], po[:])
                        nc.sync.dma_start(
                            outp[ds((b * NKB + ml) * 128, 128), :], ob[:])

                pending_T = None
                for m in range(NT):
                    b, ml = divmod(m, NKB)
                    if ml == 0:
                        QT_b = qt_pool.tile([128, GH, SQ], f16, tag="QT")
                        KT_b = kt_pool.tile([128, SQ], f16, tag="KT")
                        V_b = kt_pool.tile([128, NKB, 128], f16, tag="Vnew")
                        kcT_b = kt_pool.tile([128, CACHED], f16, tag="kcT")
                        nc.sync.dma_start(kcT_b[:], kcT[b].rearrange("p k -> p k"))
                        vc_b = kt_pool.tile([128, NKB, 128], f16, tag="vc")
                        nc.sync.dma_start(
                            vc_b[:], vc[b].rearrange("(blk p) d -> p blk d", p=128))
                        seq[b] = dict(QT=QT_b, KT=KT_b, V=V_b, kcT=kcT_b, vc=vc_b)

                    if m + 4 < NT:
                        emit_ht_dma(m + 4)
                    if m >= 2 and wdmas:
                        wdmas.pop(0)()

                    # qkv projection: out [tokens(128), features(768)]
                    ht_t = hts.pop(m)
                    ps_hi = ps.tile([128, 512], f32, tag="ps")
                    ps_lo = ps.tile([128, 512], f32, tag="ps")
                    for k in range(HID // 128):
                        nc.tensor.matmul(ps_hi[:], ht_t[:, k, :],
                                         wq_sb[:, k, 0:512],
                                         start=(k == 0), stop=(k == 31))
                        nc.tensor.matmul(ps_lo[:, 0:NF - 512], ht_t[:, k, :],
                                         wq_sb[:, k, 512:NF],
                                         start=(k == 0), stop=(k == 31))

                    if pending_T is not None:
                        emit_transposes(*pending_T)

                    # RMSNorm stats straight from PSUM
                    x2 = scratch.tile([128, NH * D], f32, tag="x2")
                    nc.scalar.square(x2[:, 0:GH * D], ps_hi[:])
                    nc.scalar.square(x2[:, GH * D:NH * D], ps_lo[:, 0:128])
                    ss = work.tile([128, NH], f32, tag="ss")
                    nc.vector.reduce_sum(out=ss[:], in_=x2[:].rearrange(
                        "p (h d) -> p h d", h=NH), axis=mybir.AxisListType.X)
                    nc.scalar.activation(ss[:], ss[:],
                                         mybir.ActivationFunctionType.Sqrt,
                                         bias=eps_sb[:], scale=1.0 / D)
                    rstd = work.tile([128, NH], f32, tag="rstd")
                    nc.vector.reciprocal(rstd[:], ss[:])
                    # normalize PSUM -> qkv_sb fp16 (q heads + k); copy v out
                    qkv_sb = work.tile([128, NH * D], f16, tag="qkv_sb", bufs=3)
                    for h5 in range(NH):
                        src_ap = ps_hi[:, ts(h5, D)] if h5 < GH else \
                            ps_lo[:, 0:128]
                        nc.vector.tensor_scalar_mul(
                            qkv_sb[:, ts(h5, D)], src_ap, rstd[:, ds(h5, 1)])
                    nc.any.tensor_copy(seq[b]["V"][:, ml, :], ps_lo[:, 128:256])

                    # partial rope (DVE, all-fp16) in place on qkv_sb
                    v3 = qkv_sb[:].rearrange("p (h d) -> p h d", h=NH)
                    x1v = v3[:, :, 0:half]
                    x2v = v3[:, :, half:R]
                    cb = cs_all[:, None, m, 0:half].to_broadcast((128, NH, half))
                    sb_ = cs_all[:, None, m, half:R].to_broadcast((128, NH, half))
                    t1 = scratch.tile([128, NH, half], f16, tag="t1")
                    t2 = scratch.tile([128, NH, half], f16, tag="t2")
                    t3 = scratch.tile([128, NH, half], f16, tag="t3")
                    t4 = scratch.tile([128, NH, half], f16, tag="t4")
                    nc.vector.tensor_tensor(t1[:], x1v, cb, mult)
                    nc.vector.tensor_tensor(t2[:], x2v, sb_, mult)
                    nc.vector.tensor_tensor(t3[:], x1v, sb_, mult)
                    nc.vector.tensor_tensor(t4[:], x2v, cb, mult)
                    nc.vector.tensor_tensor(x1v, t1[:], t2[:],
                                            mybir.AluOpType.subtract)
                    nc.vector.tensor_tensor(x2v, t3[:], t4[:], add)
                    pending_T = (b, ml, qkv_sb)

                    # per-seq pipeline: after seq b's last qkv tile, o_proj of
                    # seq b-1 (27us of PE work) hides the rope->transpose
                    # dependency chain of tile (b,3); then attention(b) runs
                    # while seq b+1's qkv norm chains occupy DVE/ACT
                    if ml == NKB - 1:
                        if b > 0:
                            emit_oproj(b - 1)
                        emit_transposes(*pending_T)
                        pending_T = None
                        emit_attention(b)

                emit_oproj(B - 1)

    nc.compile()
    return nc


def _get_compiled():
    global _COMPILED
    if _COMPILED is None:
        _COMPILED = _build()
    return _COMPILED


def _prep_inputs(hidden_states, cos, sin, positions, k_cache, v_cache, page_table,
                 cache_seqlens, cu_seqlens_q, qkv_weight, o_proj_weight,
                 q_norm_weight, k_norm_weight):
    f16 = np.float16
    pos = np.asarray(positions).reshape(B, SQ)
    assert np.array_equal(np.asarray(cache_seqlens),
                          np.full(B, CACHED, np.int32)), "cache_seqlens != CACHED"
    assert np.array_equal(np.asarray(cu_seqlens_q),
                          np.arange(B + 1, dtype=np.int64) * SQ), "cu_seqlens ragged"
    assert (pos == CACHED + np.arange(SQ)[None, :]).all(), "positions ragged"
    assert np.allclose(q_norm_weight, 1.0) and np.allclose(k_norm_weight, 1.0), \
        "non-unit norm weights unsupported"

    pt = np.asarray(page_table)
    phys = (pt[:, :, None] * PAGE + np.arange(PAGE)[None, None, :]).reshape(B, -1)
    slots = pt[np.arange(B)[:, None], pos // PAGE] * PAGE + pos % PAGE
    assert np.array_equal(slots, phys[:, CACHED:]), "non-append page layout"

    kf = np.asarray(k_cache).reshape(-1, HKV, D)
    vf = np.asarray(v_cache).reshape(-1, HKV, D)
    Kc = kf[phys[:, :CACHED]]          # [B, 512, HKV, D]
    Vc = vf[phys[:, :CACHED]]

    # cs[p, m, :] = concat(cos, sin)[token m*128+p]
    cs = np.concatenate([np.asarray(cos)[positions], np.asarray(sin)[positions]],
                        axis=1).astype(f16).reshape(T // 128, 128, R)
    cs = np.ascontiguousarray(cs.transpose(1, 0, 2))
    # hT[m, p, k*128+t] = hidden[m*128+t, k*128+p]
    hT = np.ascontiguousarray(
        np.asarray(hidden_states, dtype=f16).reshape(T // 128, 128, HID // 128, 128)
        .transpose(0, 3, 2, 1).reshape(T // 128, 128, HID))
    tri = np.where(np.arange(128)[None, :] >= np.arange(128)[:, None],
                   np.float32(0.0), np.float32(NEG))
    eye = np.eye(128, dtype=f16)

    qw = np.asarray(qkv_weight)
    ow = np.asarray(o_proj_weight)
    in_maps = []
    for c in range(NCORES):
        rows = np.concatenate([
            qw[c * GH * D:(c + 1) * GH * D],
            qw[HQ * D + c * D: HQ * D + (c + 1) * D],
            qw[HQ * D + HKV * D + c * D: HQ * D + HKV * D + (c + 1) * D],
        ], axis=0)
        in_maps.append(dict(
            hT=hT,
            wqkv=np.ascontiguousarray(rows.T, dtype=f16),
            wo=np.ascontiguousarray(ow[:, c * GH * D:(c + 1) * GH * D].T, dtype=f16),
            kcT=np.ascontiguousarray(Kc[:, :, c, :].transpose(0, 2, 1), dtype=f16),
            vc=np.ascontiguousarray(Vc[:, :, c, :], dtype=f16),
            csel=cs, trimask=tri, ident=eye,
            onesd=np.ones((128, 128), dtype=f16),
        ))
    return in_maps


def kernel(**inputs) -> np.ndarray:
    from concourse.bass_utils import run_bass_kernel_spmd
    in_maps = _prep_inputs(**inputs)
    nc = _get_compiled()
    res = run_bass_kernel_spmd(nc, in_maps, core_ids=list(range(NCORES)))
    acc = res.results[0]["outp"].astype(np.float32)
    for c in range(1, NCORES):
        acc += res.results[c]["outp"].astype(np.float32)
    return acc
